# revision 50
# baseline (speedup 1.0000x reference)
"""Multi-head causal self-attention (B=2, L=2048, D=1024, H=16) on 8 TRN2
NeuronCores.

Sharding: core c handles batch b = c // 4 and head group g = c % 4 (4 heads,
i.e. a 256-wide slice of the QKV output dim and the matching 256 rows of
Wo^T).  Each core computes a full (L, D) partial of the output projection;
the host sums the 4 partials per batch and adds bo.

v2 layout: all inputs are pre-transposed AND converted to f16 on the host
(x^T [D, L], W{q,k,v}^T [D, C], Wo_slice^T [C, D]) so no PE transposes or
PSUM->SBUF relayout casts are needed on device — phase A is pure dense
matmul.  Biases ride the PE too (rank-1 ones-row matmuls appended to each
accumulation group).  Emission interleaves the K/V/Q projections of block
qt+1 with the attention of block qt so the ACT exp stream (the co-bottleneck
at ~1.3us per [128,1024] tile) starts ~14us in and overlaps projection
matmuls.

On-core tiles (f16 unless noted):
  XT  [128, 8, 2048]   x^T (d-chunk on partitions), straight from DMA
  W*T [128, 8, 256]    W^T, straight from DMA
  WoT [128, 2, 1024]   Wo^T slice, straight from DMA
  QT  [128, 2, 512]x4  q^T (dq on partitions, chunk = head pair)
  KTz [128, 4, 512]x4  k^T zero-padded per head to K=128 rows (PE HAM clock
                       gate needs full-partition streams; K=64 is ~1.6x off)
  Vp  [128, 4, 4, 65]  v natural + ones column (softmax denominator trick)
  OT  [128, 2, 512]x4  attention out^T, normalized in place

Attention per (qt, head): s^T[k, q] = KTz_h . QT_pair; exp on ACT from a
2-bank PSUM pair; causal mask via gpsimd affine_select on diagonal tiles;
o^T + denominator accumulated in PSUM with V'; normalize = PE
ones-broadcast of the denominator + reciprocal_approx_fast (custom DVE,
~5x the iterative divide) + one multiply, emitted one tile late so the PE
stream never waits.  Output projection woven in per 512-row q block.
"""

import sys

for _p in ("/opt/trn_rl_repo", "/root/.axon_site/_ro/trn_rl_repo"):
    if _p not in sys.path:
        sys.path.append(_p)

from contextlib import ExitStack

import numpy as np

import concourse.bass as bass
import concourse.tile as tile
from concourse import bacc, mybir
from concourse.bass_utils import run_bass_kernel_spmd

F32 = mybir.dt.float32
F16 = mybir.dt.float16

B, L, D, H = 2, 2048, 1024, 16
DK = D // H  # 64
NCORES = 8
GH = 4  # heads per core
C = GH * DK  # 256: per-core slice of the qkv/head dim
QT_TILES = L // 512  # 4
DCH = D // 128  # 8


def _build_program():
    nc = bacc.Bacc("TRN2", target_bir_lowering=False, debug=False, num_devices=NCORES)

    # weights arrive chunk-major ([partition, chunk, n]) so every load is one
    # DMA of 128 dense 4KB descriptors — the [D, C] rearrange layout cost
    # ~4us of descriptor-build per DMA on the sync engine.
    xt_d = nc.dram_tensor("xt", [D, L], F16, kind="ExternalInput").ap()
    wqt_d = nc.dram_tensor("wqt", [128, DCH, C], F16, kind="ExternalInput").ap()
    wkt_d = nc.dram_tensor("wkt", [128, DCH, C], F16, kind="ExternalInput").ap()
    wvt_d = nc.dram_tensor("wvt", [128, DCH, C], F16, kind="ExternalInput").ap()
    wot_d = nc.dram_tensor("wot", [128, 2, D], F16, kind="ExternalInput").ap()
    bq_d = nc.dram_tensor("bq", [C], F32, kind="ExternalInput").ap()
    bk_d = nc.dram_tensor("bk", [C], F32, kind="ExternalInput").ap()
    bv_d = nc.dram_tensor("bv", [C], F16, kind="ExternalInput").ap()
    out_d = nc.dram_tensor("out", [L, D], F16, kind="ExternalOutput").ap()

    with tile.TileContext(nc) as tc, ExitStack() as ctx:
        pool = ctx.enter_context(tc.tile_pool(name="persist", bufs=1))
        # PSUM budget (8 banks): pss [128,1024] x2 = 4 banks (scores/proj),
        # pso [128,512] x2 (o^T accumulators), tmp [128,512] x2 (psb/psy).
        psA = ctx.enter_context(tc.tile_pool(name="psA", bufs=2, space="PSUM"))
        psB = ctx.enter_context(tc.tile_pool(name="psB", bufs=2, space="PSUM"))
        cp = ctx.enter_context(tc.tile_pool(name="copies", bufs=4))
        yp = ctx.enter_context(tc.tile_pool(name="youts", bufs=3))
        rbp = ctx.enter_context(tc.tile_pool(name="rbs", bufs=2))
        dnp = ctx.enter_context(tc.tile_pool(name="dens", bufs=2))

        ones_sb = pool.tile([128, 512], F16)
        nc.gpsimd.memset(ones_sb[:], 1.0)

        # DMA issues cost ~600-950ns each on the issuing engine, so they are
        # hand-assigned across the three DMA-capable engines (sync, scalar,
        # gpsimd) in consumer order: sync+scalar stream x^T column waves (the
        # critical path), gpsimd carries biases (tiny, needed by the first
        # PSUM->SBUF casts) and the non-K weights.
        XT = pool.tile([128, DCH, L], F16)
        WT = {
            name: pool.tile([128, DCH, C], F16, name=f"W{name}T")
            for name in ("q", "k", "v")
        }
        WoT = pool.tile([128, 2, D], F16)
        bq_sb = pool.tile([128, 2], F32)
        bk_sb = pool.tile([128, 2], F32)
        bv_sb = pool.tile([1, C], F16)

        # scalar (= ACT) gets only head-critical issues: its later exp stream
        # must not sit behind DMA-queue backpressure waits.  gpsimd carries
        # the small/early tensors its affine_selects don't need until ~14us.
        # sync absorbs the deep x^T waves (its only later duty is out-DMAs).
        for i in range(4):  # quarters: one HW queue streams only ~50GB/s
            eng = nc.sync if i % 2 == 0 else nc.scalar
            eng.dma_start(
                WT["k"][:, 2 * i : 2 * i + 2, :], wkt_d[:, 2 * i : 2 * i + 2, :]
            )
        nc.gpsimd.dma_start(bk_sb[:], bk_d.rearrange("(c p) -> p c", p=128))
        nc.gpsimd.dma_start(bq_sb[:], bq_d.rearrange("(c p) -> p c", p=128))
        nc.gpsimd.dma_start(bv_sb[:], bv_d[None, :])
        for dc in range(DCH):
            eng = nc.sync if dc % 2 == 0 else nc.scalar
            eng.dma_start(XT[:, dc, 0:512], xt_d[dc * 128 : (dc + 1) * 128, 0:512])
        for i in range(4):
            nc.gpsimd.dma_start(
                WT["q"][:, 2 * i : 2 * i + 2, :], wqt_d[:, 2 * i : 2 * i + 2, :]
            )
        for i in range(4):
            nc.gpsimd.dma_start(
                WT["v"][:, 2 * i : 2 * i + 2, :], wvt_d[:, 2 * i : 2 * i + 2, :]
            )
        for lo, hi in ((512, 1024), (1024, L)):
            for dc in range(DCH):
                nc.sync.dma_start(
                    XT[:, dc, lo:hi], xt_d[dc * 128 : (dc + 1) * 128, lo:hi]
                )
        nc.sync.dma_start(WoT[:, 0:1, :], wot_d[:, 0:1, :])
        nc.sync.dma_start(WoT[:, 1:2, :], wot_d[:, 1:2, :])

        QTs = [pool.tile([128, 2, 512], F16, name=f"QT{g}") for g in range(4)]
        KTzs = [pool.tile([128, GH, 512], F16, name=f"KTz{g}") for g in range(4)]
        for g in range(4):
            nc.gpsimd.memset(KTzs[g][:], 0.0)
        Vps = [pool.tile([128, 4, GH, DK + 1], F16, name=f"Vp{g}") for g in range(4)]
        for g in range(4):
            nc.vector.tensor_copy(Vps[g][:, :, :, DK : DK + 1], ones_sb[:, 0:16])
        OTs = [pool.tile([128, 2, 512], F16, name=f"OT{g}") for g in range(4)]
        # cj=0 output-projection partials of the LAST q-block: computed as
        # mid-attention filler once heads 0/1 are normalized, finished (cj=1
        # + add + DMA) in the tail.
        y0s = [pool.tile([128, 1024], F16, name=f"y0_{s}") for s in range(4)]

        with nc.allow_low_precision(reason="f16 activations/weights throughout"):

            def proj_k(qt, j):
                """K^T dq-chunk j for k-cols [512qt, 512qt+512) -> KTz.  The
                k bias (partition-indexed) rides the PSUM->SBUF cast as a
                broadcast add."""
                psk = psB.tile([128, 512], F32, tag="tmp", name="psk")
                for dci in range(DCH):
                    nc.tensor.matmul(
                        psk[:],
                        lhsT=WT["k"][:, dci, j * 128 : (j + 1) * 128],
                        rhs=XT[:, dci, qt * 512 : (qt + 1) * 512],
                        start=(dci == 0),
                        stop=(dci == DCH - 1),
                    )
                for half in range(2):
                    hp = 64 * half
                    nc.vector.tensor_tensor(
                        KTzs[qt][hp : hp + 64, 2 * j + half, :],
                        psk[hp : hp + 64, :],
                        bk_sb[hp : hp + 64, j, None].to_broadcast((64, 512)),
                        mybir.AluOpType.add,
                    )

            def proj_v(qt, half):
                """V natural for k-tiles [4qt+2*half, +2) -> Vp; v bias (free
                dim) rides the PE as a rank-1 ones matmul."""
                psv = psB.tile([128, 512], F32, tag="tmp", name="psv")
                for kk in range(2):
                    kt = 4 * qt + 2 * half + kk
                    sl = slice(kk * 256, (kk + 1) * 256)
                    for dci in range(DCH):
                        nc.tensor.matmul(
                            psv[:, sl],
                            lhsT=XT[:, dci, kt * 128 : (kt + 1) * 128],
                            rhs=WT["v"][:, dci, :],
                            start=(dci == 0),
                            stop=False,
                        )
                    nc.tensor.matmul(
                        psv[:, sl],
                        lhsT=ones_sb[0:1, 0:128],
                        rhs=bv_sb[:],
                        start=False,
                        stop=True,
                    )
                nc.vector.tensor_copy(
                    Vps[qt][:, 2 * half : 2 * half + 2, :, 0:DK],
                    psv[:].rearrange("p (k h d) -> p k h d", k=2, h=GH),
                )

            def proj_q(qt, j):
                psq = psB.tile([128, 512], F32, tag="tmp", name="psq")
                for dci in range(DCH):
                    nc.tensor.matmul(
                        psq[:],
                        lhsT=WT["q"][:, dci, j * 128 : (j + 1) * 128],
                        rhs=XT[:, dci, qt * 512 : (qt + 1) * 512],
                        start=(dci == 0),
                        stop=(dci == DCH - 1),
                    )
                nc.vector.tensor_tensor(
                    QTs[qt][:, j, :],
                    psq[:],
                    bq_sb[:, j, None].to_broadcast((128, 512)),
                    mybir.AluOpType.add,
                )

            def proj_parts(qt):
                return [
                    lambda qt=qt: proj_k(qt, 0),
                    lambda qt=qt: proj_k(qt, 1),
                    lambda qt=qt: proj_v(qt, 0),
                    lambda qt=qt: proj_v(qt, 1),
                    lambda qt=qt: proj_q(qt, 0),
                    lambda qt=qt: proj_q(qt, 1),
                ]

            def proj_kq0():
                """Interleaved K/Q j=0 chains for block 0: both consume x^T
                chunk dci as it lands, so the head is paced by the DMA wave
                once, not twice."""
                psk = psB.tile([128, 512], F32, tag="tmp", name="psk")
                psq = psB.tile([128, 512], F32, tag="tmp", name="psq")
                for dci in range(DCH):
                    for wt, ps in ((WT["k"], psk), (WT["q"], psq)):
                        nc.tensor.matmul(
                            ps[:],
                            lhsT=wt[:, dci, 0:128],
                            rhs=XT[:, dci, 0:512],
                            start=(dci == 0),
                            stop=(dci == DCH - 1),
                        )
                for half in range(2):
                    hp = 64 * half
                    nc.vector.tensor_tensor(
                        KTzs[0][hp : hp + 64, half, :],
                        psk[hp : hp + 64, :],
                        bk_sb[hp : hp + 64, 0, None].to_broadcast((64, 512)),
                        mybir.AluOpType.add,
                    )
                nc.vector.tensor_tensor(
                    QTs[0][:, 0, :],
                    psq[:],
                    bq_sb[:, 0, None].to_broadcast((128, 512)),
                    mybir.AluOpType.add,
                )

            def normalize(h, qt, pso):
                hj, hp = h // 2, 64 * (h % 2)
                den_r = dnp.tile([1, 512], F16, tag="den")
                nc.vector.tensor_copy(den_r[:], pso[64:65, :])
                psb = psB.tile([128, 512], F32, tag="tmp", name="psb")
                nc.tensor.matmul(
                    psb[:64],
                    lhsT=ones_sb[0:1, 0:64],
                    rhs=den_r[:],
                    start=True,
                    stop=True,
                )
                rb = rbp.tile([64, 512], F32, tag="rb")
                nc.vector.reciprocal_approx_fast(rb[:], psb[:64])
                nc.vector.tensor_tensor(
                    OTs[qt][hp : hp + 64, hj, :],
                    pso[:64],
                    rb[:],
                    mybir.AluOpType.mult,
                )

            def outproj_sub(qt512, sub):
                # project q rows [qt512*512 + 128*sub, +128) and DMA them out;
                # woven into the next q-tile's attention as PE gap-filler.
                q0 = qt512 * 512 + sub * 128
                y_sb = yp.tile([128, 1024], F16, tag="y")
                for e in range(2):
                    psy = psB.tile([128, 512], F32, tag="tmp", name="psy")
                    for cj in range(2):
                        nc.tensor.matmul(
                            psy[:],
                            lhsT=OTs[qt512][:, cj, sub * 128 : (sub + 1) * 128],
                            rhs=WoT[:, cj, e * 512 : (e + 1) * 512],
                            start=(cj == 0),
                            stop=(cj == 1),
                        )
                    nc.vector.tensor_copy(y_sb[:, e * 512 : (e + 1) * 512], psy[:])
                    nc.sync.dma_start(
                        out_d[q0 : q0 + 128, e * 512 : (e + 1) * 512],
                        y_sb[:, e * 512 : (e + 1) * 512],
                    )

            def outproj_cj0(sub):
                lq = QT_TILES - 1
                for e in range(2):
                    psy = psB.tile([128, 512], F32, tag="tmp", name="psy")
                    nc.tensor.matmul(
                        psy[:],
                        lhsT=OTs[lq][:, 0, sub * 128 : (sub + 1) * 128],
                        rhs=WoT[:, 0, e * 512 : (e + 1) * 512],
                        start=True,
                        stop=True,
                    )
                    nc.vector.tensor_copy(y0s[sub][:, e * 512 : (e + 1) * 512], psy[:])

            def outproj_cj1(sub):
                lq = QT_TILES - 1
                q0 = lq * 512 + sub * 128
                y_sb = yp.tile([128, 1024], F16, tag="y")
                for e in range(2):
                    psy = psB.tile([128, 512], F32, tag="tmp", name="psy")
                    nc.tensor.matmul(
                        psy[:],
                        lhsT=OTs[lq][:, 1, sub * 128 : (sub + 1) * 128],
                        rhs=WoT[:, 1, e * 512 : (e + 1) * 512],
                        start=True,
                        stop=True,
                    )
                    nc.vector.tensor_tensor(
                        y_sb[:, e * 512 : (e + 1) * 512],
                        psy[:],
                        y0s[sub][:, e * 512 : (e + 1) * 512],
                        mybir.AluOpType.add,
                    )
                    nc.sync.dma_start(
                        out_d[q0 : q0 + 128, e * 512 : (e + 1) * 512],
                        y_sb[:, e * 512 : (e + 1) * 512],
                    )

            def av_group(qt, h, entries, pso, p_sb, n_kt):
                """Causal mask + o^T accumulation for one packed score group;
                emitted one group late so the PE stream never waits on the
                exp.  `entries` = [(kt, q0, dst)]: k-tile kt covers query cols
                [q0, 512) of the block, packed at p_sb column dst."""
                for kt, q0, dst in entries:
                    w = 512 - q0
                    if kt >= 4 * qt:  # diagonal overlap: causal mask
                        nc.gpsimd.affine_select(
                            out=p_sb[:, dst : dst + w],
                            in_=p_sb[:, dst : dst + w],
                            pattern=[[1, w]],
                            compare_op=mybir.AluOpType.is_ge,
                            fill=0.0,
                            base=qt * 512 + q0 - kt * 128,
                            channel_multiplier=-1,
                        )
                    nc.tensor.matmul(
                        pso[:65, q0:512],
                        lhsT=Vps[kt // 4][:, kt % 4, h, :],
                        rhs=p_sb[:, dst : dst + w],
                        start=(kt == 0),
                        stop=(kt == n_kt - 1),
                        skip_group_check=True,
                    )

            pending = None
            # qt=0 head: heads 0/1 only need the j=0 chunk of Q^T/K^T and
            # the first exp needs no V at all — emit the bare minimum and
            # push V / the j=1 chunks into the first h-loops' slots.
            proj_kq0()
            for qt in range(QT_TILES):
                # projections for the NEXT block are woven into this block's
                # attention at h-loop boundaries: the attention stretch is
                # exp-paced on ACT, so the proj matmuls ride in PE's slack
                # instead of forming their own ACT-idle stretch.
                n_groups = 2 * qt + 2
                parts, pops, sched = [], [0, 0, 0, 0], {}
                if qt == 0:
                    # V rides just behind the first exps (AV needs it one
                    # group later); j=1 chunks land before h2 needs them.
                    sched[(0, 0)] = [lambda: proj_v(0, 0)]
                    sched[(0, 1)] = [lambda: proj_v(0, 1)]
                    sched[(1, 0)] = [lambda: proj_k(0, 1)]
                    sched[(1, 1)] = [lambda: proj_q(0, 1)]
                    pp1 = proj_parts(1)
                    sched[(2, 0)] = pp1[0:2]
                    sched[(2, 1)] = pp1[2:3]
                    sched[(3, 0)] = pp1[3:5]
                    sched[(3, 1)] = pp1[5:6]
                elif qt < QT_TILES - 1:
                    # interleave next block's projections with the previous
                    # block's output projection in mid-loop slots, where the
                    # 2-deep exp queue keeps ACT fed through each PE detour.
                    # K/Q chains first — they gate the next block's scores;
                    # V is only needed by its AV one group in.
                    pp = proj_parts(qt + 1)
                    ou = [lambda s=i: outproj_sub(qt - 1, s) for i in range(4)]
                    fill = [
                        pp[0], pp[4], ou[0], pp[1], pp[5], ou[1],
                        pp[2], ou[2], pp[3], ou[3],
                    ]
                    slots = [
                        (h, g)
                        for h in range(GH)
                        for g in range(1, n_groups, 2)
                    ]
                    for i, fn in enumerate(fill):
                        sched.setdefault(slots[i % len(slots)], []).append(fn)
                else:
                    # last block: previous block's outproj in h0/h1 slots,
                    # this block's cj=0 outproj partials in h2/h3 (heads 0/1
                    # are normalized by then)
                    for i, sl in enumerate([(0, 1), (0, 5), (1, 1), (1, 5)]):
                        sched.setdefault(sl, []).append(
                            lambda s=i: outproj_sub(qt - 1, s)
                        )
                    for i, sl in enumerate([(2, 1), (2, 5), (3, 1), (3, 5)]):
                        sched.setdefault(sl, []).append(lambda s=i: outproj_cj0(s))
                n_kt = 4 * qt + 4
                # packed score groups: full k-tile pairs below the diagonal
                # band, then the band packed to its valid q-suffixes
                # (512+384 and 256+128 cols) so exp/AV skip the masked half.
                groups = [
                    [(2 * g, 0, 0), (2 * g + 1, 0, 512)] for g in range(2 * qt)
                ]
                groups.append([(4 * qt, 0, 0), (4 * qt + 1, 128, 512)])
                groups.append([(4 * qt + 2, 256, 0), (4 * qt + 3, 384, 256)])
                for h in range(GH):
                    hj = h // 2
                    pso = psB.tile([128, 512], F32, tag="pso")
                    prevq = []  # AV rides TWO groups behind the exp so the
                    # in-order PE stream clears the exp+affine latency
                    for g, entries in enumerate(groups):
                        pss = psA.tile([128, 1024], F32, tag="pss", name="pss")
                        for kt, q0, dst in entries:
                            nc.tensor.matmul(
                                pss[:, dst : dst + 512 - q0],
                                lhsT=KTzs[kt // 4][
                                    :, h, (kt % 4) * 128 : (kt % 4 + 1) * 128
                                ],
                                rhs=QTs[qt][:, hj, q0:512],
                                start=True,
                                stop=True,
                            )
                        width = entries[-1][2] + 512 - entries[-1][1]
                        p_sb = cp.tile([128, 1024], F16, tag="p", bufs=5)
                        nc.scalar.activation(
                            p_sb[:, 0:width],
                            pss[:, 0:width],
                            mybir.ActivationFunctionType.Exp,
                            scale=0.125,
                        )
                        for fn in sched.get((h, g), []):
                            fn()  # fillers ride between exp and delayed AV
                        if prevq:
                            pe, pp = prevq.pop(0)
                            av_group(qt, h, pe, pso, pp, n_kt)
                        prevq.append((entries, p_sb))
                        if g == 0 and pending is not None:
                            normalize(*pending)  # previous tile, PE has work
                            pending = None
                    while prevq:
                        pe, pp = prevq.pop(0)
                        av_group(qt, h, pe, pso, pp, n_kt)
                    pending = (h, qt, pso)
                    for _ in range(pops[h]):
                        if parts:
                            parts.pop(0)()
            normalize(*pending)
            for sub in range(4):
                outproj_cj1(sub)

    nc.compile()
    return nc


_NC_CACHE = None


def _get_program():
    global _NC_CACHE
    if _NC_CACHE is None:
        _NC_CACHE = _build_program()
    return _NC_CACHE


def _run(in_maps, trace=False, **kw):
    nc = _get_program()
    return run_bass_kernel_spmd(nc, in_maps, list(range(NCORES)), trace=trace, **kw)


def _make_in_maps(x, Wq, bq, Wk, bk, Wv, bv, Wo, bo):
    f16 = lambda v: np.ascontiguousarray(np.asarray(v, dtype=np.float32)).astype(
        np.float16
    )

    def chunk_major(wT, nch):  # [nch*128, n] -> [128, nch, n] (partition-major)
        n = wT.shape[1]
        return np.ascontiguousarray(
            wT.reshape(nch, 128, n).transpose(1, 0, 2)
        ).astype(np.float16)

    x = np.asarray(x, dtype=np.float32)
    in_maps = []
    for core in range(NCORES):
        b, g = divmod(core, 4)
        s = slice(g * C, (g + 1) * C)
        in_maps.append(
            {
                "xt": f16(x[b].T),
                "wqt": chunk_major(np.asarray(Wq[s, :].T, np.float32), DCH),
                "wkt": chunk_major(np.asarray(Wk[s, :].T, np.float32), DCH),
                "wvt": chunk_major(np.asarray(Wv[s, :].T, np.float32), DCH),
                "wot": chunk_major(np.asarray(Wo[:, s].T, np.float32), 2),
                "bq": np.ascontiguousarray(np.asarray(bq[s], dtype=np.float32)),
                "bk": np.ascontiguousarray(np.asarray(bk[s], dtype=np.float32)),
                "bv": f16(bv[s]),
            }
        )
    return in_maps


def kernel(x, Wq, bq, Wk, bk, Wv, bv, Wo, bo, _trace=False, _trace_out=None, _tmpdir=None):
    in_maps = _make_in_maps(x, Wq, bq, Wk, bk, Wv, bv, Wo, bo)
    res = _run(in_maps, trace=_trace, tmpdir=_tmpdir)
    if _trace_out is not None:
        _trace_out.append(res)
    bo = np.asarray(bo, dtype=np.float32)
    out = np.empty((B, L, D), dtype=np.float32)
    for b in range(B):
        acc = res.results[4 * b]["out"].astype(np.float32)
        for g in range(1, 4):
            acc = acc + res.results[4 * b + g]["out"].astype(np.float32)
        out[b] = acc + bo[None, :]
    return out


# revision 51
# speedup vs baseline: 1.1825x; 1.1825x over previous
"""Multi-head causal self-attention (B=2, L=2048, D=1024, H=16) on 8 TRN2
NeuronCores.

Sharding: core c handles batch b = c // 4 and head group g = c % 4 (4 heads,
i.e. a 256-wide slice of the QKV output dim and the matching 256 rows of
Wo^T).  Each core computes a full (L, D) partial of the output projection;
the host sums the 4 partials per batch and adds bo.

v2 layout: all inputs are pre-transposed AND converted to f16 on the host
(x^T [D, L], W{q,k,v}^T [D, C], Wo_slice^T [C, D]) so no PE transposes or
PSUM->SBUF relayout casts are needed on device — phase A is pure dense
matmul.  Biases ride the PE too (rank-1 ones-row matmuls appended to each
accumulation group).  Emission interleaves the K/V/Q projections of block
qt+1 with the attention of block qt so the ACT exp stream (the co-bottleneck
at ~1.3us per [128,1024] tile) starts ~14us in and overlaps projection
matmuls.

On-core tiles (f16 unless noted):
  XT  [128, 8, 2048]   x^T (d-chunk on partitions), straight from DMA
  W*T [128, 8, 256]    W^T, straight from DMA
  WoT [128, 2, 1024]   Wo^T slice, straight from DMA
  QT  [128, 2, 512]x4  q^T (dq on partitions, chunk = head pair)
  KTz [128, 4, 512]x4  k^T zero-padded per head to K=128 rows (PE HAM clock
                       gate needs full-partition streams; K=64 is ~1.6x off)
  Vp  [128, 4, 4, 65]  v natural + ones column (softmax denominator trick)
  OT  [128, 2, 512]x4  attention out^T, normalized in place

Attention per (qt, head): s^T[k, q] = KTz_h . QT_pair; exp on ACT from a
2-bank PSUM pair; causal mask via gpsimd affine_select on diagonal tiles;
o^T + denominator accumulated in PSUM with V'; normalize = PE
ones-broadcast of the denominator + reciprocal_approx_fast (custom DVE,
~5x the iterative divide) + one multiply, emitted one tile late so the PE
stream never waits.  Output projection woven in per 512-row q block.
"""

import sys

for _p in ("/opt/trn_rl_repo", "/root/.axon_site/_ro/trn_rl_repo"):
    if _p not in sys.path:
        sys.path.append(_p)

from contextlib import ExitStack

import numpy as np

import concourse.bass as bass
import concourse.tile as tile
from concourse import bacc, mybir
from concourse.bass_utils import run_bass_kernel_spmd

F32 = mybir.dt.float32
F16 = mybir.dt.float16

B, L, D, H = 2, 2048, 1024, 16
DK = D // H  # 64
NCORES = 8
GH = 4  # heads per core
C = GH * DK  # 256: per-core slice of the qkv/head dim
QT_TILES = L // 512  # 4
DCH = D // 128  # 8


def _build_program():
    nc = bacc.Bacc("TRN2", target_bir_lowering=False, debug=False, num_devices=NCORES)

    # weights arrive chunk-major ([partition, chunk, n]) so every load is one
    # DMA of 128 dense 4KB descriptors — the [D, C] rearrange layout cost
    # ~4us of descriptor-build per DMA on the sync engine.
    xt_d = nc.dram_tensor("xt", [D, L], F16, kind="ExternalInput").ap()
    wqt_d = nc.dram_tensor("wqt", [128, DCH, C], F16, kind="ExternalInput").ap()
    wkt_d = nc.dram_tensor("wkt", [128, DCH, C], F16, kind="ExternalInput").ap()
    wvt_d = nc.dram_tensor("wvt", [128, DCH, C], F16, kind="ExternalInput").ap()
    wot_d = nc.dram_tensor("wot", [128, 2, D], F16, kind="ExternalInput").ap()
    bq_d = nc.dram_tensor("bq", [C], F32, kind="ExternalInput").ap()
    bk_d = nc.dram_tensor("bk", [C], F32, kind="ExternalInput").ap()
    bv_d = nc.dram_tensor("bv", [C], F16, kind="ExternalInput").ap()
    out_d = nc.dram_tensor("out", [L, D], F16, kind="ExternalOutput").ap()

    with tile.TileContext(nc) as tc, ExitStack() as ctx:
        pool = ctx.enter_context(tc.tile_pool(name="persist", bufs=1))
        # PSUM budget (8 banks): pss [128,1024] x2 = 4 banks (scores/proj),
        # pso [128,512] x2 (o^T accumulators), tmp [128,512] x2 (psb/psy).
        psA = ctx.enter_context(tc.tile_pool(name="psA", bufs=2, space="PSUM"))
        psB = ctx.enter_context(tc.tile_pool(name="psB", bufs=2, space="PSUM"))
        cp = ctx.enter_context(tc.tile_pool(name="copies", bufs=4))
        yp = ctx.enter_context(tc.tile_pool(name="youts", bufs=3))
        rbp = ctx.enter_context(tc.tile_pool(name="rbs", bufs=2))
        dnp = ctx.enter_context(tc.tile_pool(name="dens", bufs=2))

        ones_sb = pool.tile([128, 512], F16)
        nc.gpsimd.memset(ones_sb[:], 1.0)

        # DMA issues cost ~600-950ns each on the issuing engine, so they are
        # hand-assigned across the three DMA-capable engines (sync, scalar,
        # gpsimd) in consumer order: sync+scalar stream x^T column waves (the
        # critical path), gpsimd carries biases (tiny, needed by the first
        # PSUM->SBUF casts) and the non-K weights.
        XT = pool.tile([128, DCH, L], F16)
        WT = {
            name: pool.tile([128, DCH, C], F16, name=f"W{name}T")
            for name in ("q", "k", "v")
        }
        WoT = pool.tile([128, 2, D], F16)
        bq_sb = pool.tile([128, 2], F32)
        bk_sb = pool.tile([128, 2], F32)
        bv_sb = pool.tile([1, C], F16)

        # scalar (= ACT) gets only head-critical issues: its later exp stream
        # must not sit behind DMA-queue backpressure waits.  gpsimd carries
        # the small/early tensors its affine_selects don't need until ~14us.
        # sync absorbs the deep x^T waves (its only later duty is out-DMAs).
        for i in range(4):  # quarters: one HW queue streams only ~50GB/s
            eng = nc.sync if i % 2 == 0 else nc.scalar
            eng.dma_start(
                WT["k"][:, 2 * i : 2 * i + 2, :], wkt_d[:, 2 * i : 2 * i + 2, :]
            )
        nc.gpsimd.dma_start(bk_sb[:], bk_d.rearrange("(c p) -> p c", p=128))
        nc.gpsimd.dma_start(bq_sb[:], bq_d.rearrange("(c p) -> p c", p=128))
        nc.gpsimd.dma_start(bv_sb[:], bv_d[None, :])
        for dc in range(DCH):
            eng = nc.sync if dc % 2 == 0 else nc.scalar
            eng.dma_start(XT[:, dc, 0:512], xt_d[dc * 128 : (dc + 1) * 128, 0:512])
        for i in range(4):
            nc.gpsimd.dma_start(
                WT["q"][:, 2 * i : 2 * i + 2, :], wqt_d[:, 2 * i : 2 * i + 2, :]
            )
        for i in range(4):
            nc.gpsimd.dma_start(
                WT["v"][:, 2 * i : 2 * i + 2, :], wvt_d[:, 2 * i : 2 * i + 2, :]
            )
        for lo, hi in ((512, 1024), (1024, L)):
            for dc in range(DCH):
                nc.sync.dma_start(
                    XT[:, dc, lo:hi], xt_d[dc * 128 : (dc + 1) * 128, lo:hi]
                )
        nc.sync.dma_start(WoT[:, 0:1, :], wot_d[:, 0:1, :])
        nc.sync.dma_start(WoT[:, 1:2, :], wot_d[:, 1:2, :])

        QTs = [pool.tile([128, 2, 512], F16, name=f"QT{g}") for g in range(4)]
        KTzs = [pool.tile([128, GH, 512], F16, name=f"KTz{g}") for g in range(4)]
        for g in range(4):
            nc.gpsimd.memset(KTzs[g][:], 0.0)
        Vps = [pool.tile([128, 4, GH, DK + 1], F16, name=f"Vp{g}") for g in range(4)]
        for g in range(4):
            nc.vector.tensor_copy(Vps[g][:, :, :, DK : DK + 1], ones_sb[:, 0:16])
        OTs = [pool.tile([128, 2, 512], F16, name=f"OT{g}") for g in range(4)]
        # cj=0 output-projection partials of the LAST q-block: computed as
        # mid-attention filler once heads 0/1 are normalized, finished (cj=1
        # + add + DMA) in the tail.
        y0s = [pool.tile([128, 1024], F16, name=f"y0_{s}") for s in range(4)]

        with nc.allow_low_precision(reason="f16 activations/weights throughout"):

            def proj_k(qt, j):
                """K^T dq-chunk j for k-cols [512qt, 512qt+512) -> KTz.  The
                k bias (partition-indexed) rides the PSUM->SBUF cast as a
                broadcast add."""
                psk = psB.tile([128, 512], F32, tag="tmp", name="psk")
                for dci in range(DCH):
                    nc.tensor.matmul(
                        psk[:],
                        lhsT=WT["k"][:, dci, j * 128 : (j + 1) * 128],
                        rhs=XT[:, dci, qt * 512 : (qt + 1) * 512],
                        start=(dci == 0),
                        stop=(dci == DCH - 1),
                    )
                for half in range(2):
                    hp = 64 * half
                    nc.vector.tensor_tensor(
                        KTzs[qt][hp : hp + 64, 2 * j + half, :],
                        psk[hp : hp + 64, :],
                        bk_sb[hp : hp + 64, j, None].to_broadcast((64, 512)),
                        mybir.AluOpType.add,
                    )

            def proj_v(qt, half):
                """V natural for k-tiles [4qt+2*half, +2) -> Vp; v bias (free
                dim) rides the PE as a rank-1 ones matmul."""
                psv = psB.tile([128, 512], F32, tag="tmp", name="psv")
                for kk in range(2):
                    kt = 4 * qt + 2 * half + kk
                    sl = slice(kk * 256, (kk + 1) * 256)
                    for dci in range(DCH):
                        nc.tensor.matmul(
                            psv[:, sl],
                            lhsT=XT[:, dci, kt * 128 : (kt + 1) * 128],
                            rhs=WT["v"][:, dci, :],
                            start=(dci == 0),
                            stop=False,
                        )
                    nc.tensor.matmul(
                        psv[:, sl],
                        lhsT=ones_sb[0:1, 0:128],
                        rhs=bv_sb[:],
                        start=False,
                        stop=True,
                    )
                nc.vector.tensor_copy(
                    Vps[qt][:, 2 * half : 2 * half + 2, :, 0:DK],
                    psv[:].rearrange("p (k h d) -> p k h d", k=2, h=GH),
                )

            def proj_q(qt, j):
                psq = psB.tile([128, 512], F32, tag="tmp", name="psq")
                for dci in range(DCH):
                    nc.tensor.matmul(
                        psq[:],
                        lhsT=WT["q"][:, dci, j * 128 : (j + 1) * 128],
                        rhs=XT[:, dci, qt * 512 : (qt + 1) * 512],
                        start=(dci == 0),
                        stop=(dci == DCH - 1),
                    )
                nc.vector.tensor_tensor(
                    QTs[qt][:, j, :],
                    psq[:],
                    bq_sb[:, j, None].to_broadcast((128, 512)),
                    mybir.AluOpType.add,
                )

            def proj_parts(qt):
                return [
                    lambda qt=qt: proj_k(qt, 0),
                    lambda qt=qt: proj_k(qt, 1),
                    lambda qt=qt: proj_v(qt, 0),
                    lambda qt=qt: proj_v(qt, 1),
                    lambda qt=qt: proj_q(qt, 0),
                    lambda qt=qt: proj_q(qt, 1),
                ]

            def proj_kq0():
                """Interleaved K/Q j=0 chains for block 0: both consume x^T
                chunk dci as it lands, so the head is paced by the DMA wave
                once, not twice."""
                psk = psB.tile([128, 512], F32, tag="tmp", name="psk")
                psq = psB.tile([128, 512], F32, tag="tmp", name="psq")
                for dci in range(DCH):
                    for wt, ps in ((WT["k"], psk), (WT["q"], psq)):
                        nc.tensor.matmul(
                            ps[:],
                            lhsT=wt[:, dci, 0:128],
                            rhs=XT[:, dci, 0:512],
                            start=(dci == 0),
                            stop=(dci == DCH - 1),
                        )
                for half in range(2):
                    hp = 64 * half
                    nc.vector.tensor_tensor(
                        KTzs[0][hp : hp + 64, half, :],
                        psk[hp : hp + 64, :],
                        bk_sb[hp : hp + 64, 0, None].to_broadcast((64, 512)),
                        mybir.AluOpType.add,
                    )
                nc.vector.tensor_tensor(
                    QTs[0][:, 0, :],
                    psq[:],
                    bq_sb[:, 0, None].to_broadcast((128, 512)),
                    mybir.AluOpType.add,
                )

            def normalize(h, qt, pso):
                hj, hp = h // 2, 64 * (h % 2)
                den_r = dnp.tile([1, 512], F16, tag="den")
                nc.vector.tensor_copy(den_r[:], pso[64:65, :])
                psb = psB.tile([128, 512], F32, tag="tmp", name="psb")
                nc.tensor.matmul(
                    psb[:64],
                    lhsT=ones_sb[0:1, 0:64],
                    rhs=den_r[:],
                    start=True,
                    stop=True,
                )
                rb = rbp.tile([64, 512], F32, tag="rb")
                nc.vector.reciprocal_approx_fast(rb[:], psb[:64])
                nc.vector.tensor_tensor(
                    OTs[qt][hp : hp + 64, hj, :],
                    pso[:64],
                    rb[:],
                    mybir.AluOpType.mult,
                )

            def outproj_sub(qt512, sub):
                # project q rows [qt512*512 + 128*sub, +128) and DMA them out;
                # woven into the next q-tile's attention as PE gap-filler.
                q0 = qt512 * 512 + sub * 128
                y_sb = yp.tile([128, 1024], F16, tag="y")
                for e in range(2):
                    psy = psB.tile([128, 512], F32, tag="tmp", name="psy")
                    for cj in range(2):
                        nc.tensor.matmul(
                            psy[:],
                            lhsT=OTs[qt512][:, cj, sub * 128 : (sub + 1) * 128],
                            rhs=WoT[:, cj, e * 512 : (e + 1) * 512],
                            start=(cj == 0),
                            stop=(cj == 1),
                        )
                    nc.vector.tensor_copy(y_sb[:, e * 512 : (e + 1) * 512], psy[:])
                    nc.sync.dma_start(
                        out_d[q0 : q0 + 128, e * 512 : (e + 1) * 512],
                        y_sb[:, e * 512 : (e + 1) * 512],
                    )

            def outproj_cj0(sub):
                lq = QT_TILES - 1
                for e in range(2):
                    psy = psB.tile([128, 512], F32, tag="tmp", name="psy")
                    nc.tensor.matmul(
                        psy[:],
                        lhsT=OTs[lq][:, 0, sub * 128 : (sub + 1) * 128],
                        rhs=WoT[:, 0, e * 512 : (e + 1) * 512],
                        start=True,
                        stop=True,
                    )
                    nc.vector.tensor_copy(y0s[sub][:, e * 512 : (e + 1) * 512], psy[:])

            def outproj_cj1(sub):
                lq = QT_TILES - 1
                q0 = lq * 512 + sub * 128
                y_sb = yp.tile([128, 1024], F16, tag="y")
                for e in range(2):
                    psy = psB.tile([128, 512], F32, tag="tmp", name="psy")
                    nc.tensor.matmul(
                        psy[:],
                        lhsT=OTs[lq][:, 1, sub * 128 : (sub + 1) * 128],
                        rhs=WoT[:, 1, e * 512 : (e + 1) * 512],
                        start=True,
                        stop=True,
                    )
                    nc.vector.tensor_tensor(
                        y_sb[:, e * 512 : (e + 1) * 512],
                        psy[:],
                        y0s[sub][:, e * 512 : (e + 1) * 512],
                        mybir.AluOpType.add,
                    )
                    nc.sync.dma_start(
                        out_d[q0 : q0 + 128, e * 512 : (e + 1) * 512],
                        y_sb[:, e * 512 : (e + 1) * 512],
                    )

            def av_group(qt, h, entries, pso, p_sb, n_kt):
                """Causal mask + o^T accumulation for one packed score group;
                emitted one group late so the PE stream never waits on the
                exp.  `entries` = [(kt, q0, dst)]: k-tile kt covers query cols
                [q0, 512) of the block, packed at p_sb column dst."""
                for kt, q0, dst in entries:
                    w = 512 - q0
                    if kt >= 4 * qt:  # diagonal overlap: causal mask
                        nc.gpsimd.affine_select(
                            out=p_sb[:, dst : dst + w],
                            in_=p_sb[:, dst : dst + w],
                            pattern=[[1, w]],
                            compare_op=mybir.AluOpType.is_ge,
                            fill=0.0,
                            base=qt * 512 + q0 - kt * 128,
                            channel_multiplier=-1,
                        )
                    nc.tensor.matmul(
                        pso[:65, q0:512],
                        lhsT=Vps[kt // 4][:, kt % 4, h, :],
                        rhs=p_sb[:, dst : dst + w],
                        start=(kt == 0),
                        stop=(kt == n_kt - 1),
                        skip_group_check=True,
                    )

            pending = None
            # qt=0 head: heads 0/1 only need the j=0 chunk of Q^T/K^T and
            # the first exp needs no V at all — emit the bare minimum and
            # push V / the j=1 chunks into the first h-loops' slots.
            proj_kq0()
            for qt in range(QT_TILES):
                # projections for the NEXT block are woven into this block's
                # attention at h-loop boundaries: the attention stretch is
                # exp-paced on ACT, so the proj matmuls ride in PE's slack
                # instead of forming their own ACT-idle stretch.
                n_groups = 2 * qt + 2
                parts, pops, sched = [], [0, 0, 0, 0], {}
                if qt == 0:
                    # V rides just behind the first exps (AV needs it one
                    # group later); j=1 chunks land before h2 needs them.
                    sched[(0, 0)] = [lambda: proj_v(0, 0)]
                    sched[(0, 1)] = [lambda: proj_v(0, 1)]
                    sched[(1, 0)] = [lambda: proj_k(0, 1)]
                    sched[(1, 1)] = [lambda: proj_q(0, 1)]
                    pp1 = proj_parts(1)
                    sched[(2, 0)] = pp1[0:2]
                    sched[(2, 1)] = pp1[2:3]
                    sched[(3, 0)] = pp1[3:5]
                    sched[(3, 1)] = pp1[5:6]
                elif qt < QT_TILES - 1:
                    # interleave next block's projections with the previous
                    # block's output projection in mid-loop slots, where the
                    # 2-deep exp queue keeps ACT fed through each PE detour.
                    # K/Q chains first — they gate the next block's scores;
                    # V (consumed by AV one group in) can trail.
                    pp = proj_parts(qt + 1)
                    pp = [pp[0], pp[1], pp[4], pp[5], pp[2], pp[3]]
                    fill = []
                    for i in range(4):
                        fill.append(pp[i])
                        fill.append(lambda s=i: outproj_sub(qt - 1, s))
                    fill += pp[4:]
                    slots = [
                        (h, g)
                        for h in range(GH)
                        for g in range(1, n_groups, 2)
                    ]
                    slots.sort(key=lambda s: (s[1] != 1, s[0], s[1]))
                    for fn, sl in zip(fill, slots):
                        sched.setdefault(sl, []).append(fn)
                    for fn in fill[len(slots) :]:
                        sched.setdefault((GH - 1, n_groups - 1), []).append(fn)
                else:
                    # last block: previous block's outproj in h0/h1 slots,
                    # this block's cj=0 outproj partials in h2/h3 (heads 0/1
                    # are normalized by then)
                    for i, sl in enumerate([(0, 1), (0, 5), (1, 1), (1, 5)]):
                        sched.setdefault(sl, []).append(
                            lambda s=i: outproj_sub(qt - 1, s)
                        )
                    for i, sl in enumerate([(2, 1), (2, 5), (3, 1), (3, 5)]):
                        sched.setdefault(sl, []).append(lambda s=i: outproj_cj0(s))
                n_kt = 4 * qt + 4
                # packed score groups: full k-tile pairs below the diagonal
                # band, then the band packed to its valid q-suffixes
                # (512+384 and 256+128 cols) so exp/AV skip the masked half.
                groups = [
                    [(2 * g, 0, 0), (2 * g + 1, 0, 512)] for g in range(2 * qt)
                ]
                groups.append([(4 * qt, 0, 0), (4 * qt + 1, 128, 512)])
                groups.append([(4 * qt + 2, 256, 0), (4 * qt + 3, 384, 256)])
                for h in range(GH):
                    hj = h // 2
                    pso = psB.tile([128, 512], F32, tag="pso")
                    prevq = []  # AV rides TWO groups behind the exp so the
                    # in-order PE stream clears the exp+affine latency
                    for g, entries in enumerate(groups):
                        pss = psA.tile([128, 1024], F32, tag="pss", name="pss")
                        for kt, q0, dst in entries:
                            nc.tensor.matmul(
                                pss[:, dst : dst + 512 - q0],
                                lhsT=KTzs[kt // 4][
                                    :, h, (kt % 4) * 128 : (kt % 4 + 1) * 128
                                ],
                                rhs=QTs[qt][:, hj, q0:512],
                                start=True,
                                stop=True,
                            )
                        width = entries[-1][2] + 512 - entries[-1][1]
                        p_sb = cp.tile([128, 1024], F16, tag="p", bufs=5)
                        nc.scalar.activation(
                            p_sb[:, 0:width],
                            pss[:, 0:width],
                            mybir.ActivationFunctionType.Exp,
                            scale=0.125,
                        )
                        for fn in sched.get((h, g), []):
                            fn()  # fillers ride between exp and delayed AV
                        if prevq:
                            pe, pp = prevq.pop(0)
                            av_group(qt, h, pe, pso, pp, n_kt)
                        prevq.append((entries, p_sb))
                        if g == 0 and pending is not None:
                            normalize(*pending)  # previous tile, PE has work
                            pending = None
                    while prevq:
                        pe, pp = prevq.pop(0)
                        av_group(qt, h, pe, pso, pp, n_kt)
                    pending = (h, qt, pso)
                    for _ in range(pops[h]):
                        if parts:
                            parts.pop(0)()
            normalize(*pending)
            for sub in range(4):
                outproj_cj1(sub)

    nc.compile()
    return nc


_NC_CACHE = None


def _get_program():
    global _NC_CACHE
    if _NC_CACHE is None:
        _NC_CACHE = _build_program()
    return _NC_CACHE


def _run(in_maps, trace=False, **kw):
    nc = _get_program()
    return run_bass_kernel_spmd(nc, in_maps, list(range(NCORES)), trace=trace, **kw)


def _make_in_maps(x, Wq, bq, Wk, bk, Wv, bv, Wo, bo):
    f16 = lambda v: np.ascontiguousarray(np.asarray(v, dtype=np.float32)).astype(
        np.float16
    )

    def chunk_major(wT, nch):  # [nch*128, n] -> [128, nch, n] (partition-major)
        n = wT.shape[1]
        return np.ascontiguousarray(
            wT.reshape(nch, 128, n).transpose(1, 0, 2)
        ).astype(np.float16)

    x = np.asarray(x, dtype=np.float32)
    in_maps = []
    for core in range(NCORES):
        b, g = divmod(core, 4)
        s = slice(g * C, (g + 1) * C)
        in_maps.append(
            {
                "xt": f16(x[b].T),
                "wqt": chunk_major(np.asarray(Wq[s, :].T, np.float32), DCH),
                "wkt": chunk_major(np.asarray(Wk[s, :].T, np.float32), DCH),
                "wvt": chunk_major(np.asarray(Wv[s, :].T, np.float32), DCH),
                "wot": chunk_major(np.asarray(Wo[:, s].T, np.float32), 2),
                "bq": np.ascontiguousarray(np.asarray(bq[s], dtype=np.float32)),
                "bk": np.ascontiguousarray(np.asarray(bk[s], dtype=np.float32)),
                "bv": f16(bv[s]),
            }
        )
    return in_maps


def kernel(x, Wq, bq, Wk, bk, Wv, bv, Wo, bo, _trace=False, _trace_out=None, _tmpdir=None):
    in_maps = _make_in_maps(x, Wq, bq, Wk, bk, Wv, bv, Wo, bo)
    res = _run(in_maps, trace=_trace, tmpdir=_tmpdir)
    if _trace_out is not None:
        _trace_out.append(res)
    bo = np.asarray(bo, dtype=np.float32)
    out = np.empty((B, L, D), dtype=np.float32)
    for b in range(B):
        acc = res.results[4 * b]["out"].astype(np.float32)
        for g in range(1, 4):
            acc = acc + res.results[4 * b + g]["out"].astype(np.float32)
        out[b] = acc + bo[None, :]
    return out
